# revision 10
# baseline (speedup 1.0000x reference)
"""ContrastiveLoss Trainium2 kernel.

Strategy (data-parallel over 8 NeuronCores):
  - 8 cores = 4 batches x 2 halves. Core c handles batch b=c//2, half h=c%2:
    2500 match pairs + 25000 non-match pairs.
  - Gather primitive: SWDGE vector-indirect DMA (`indirect_dma_start`), which
    on TRN2 fetches ONE dynamic row index per SBUF partition and streams the
    row (16 f32 = 64 B) into that partition. One instruction = 128 rows.
    Samples are column-blocked: sample s = block j * 128 + partition p, so
    block j's indices live in idx[:, j] and its rows land in g[:, 16j:16j+16].
  - Per-sample math on DVE/ACT (fully hidden under the gather stream):
      match partial  = sum((mA-mB)^2)              (DVE sub + fused sq-reduce)
      nonmatch partial = sum(relu(0.5-||nA-nB||^2)) (DVE sub, ACT square,
        DVE grouped reduce over D=16, ACT relu with fused accumulate)
  - Padding: tail samples use index 0 on both sides; a host-built {0,1} mask
    zeroes pad match diffs exactly, and a host-built additive bias pushes pad
    nonmatch distances to 1e9 so the hinge is exactly 0.
  - Partition reduction via a ones-vector TensorE matmul into PSUM.
  - Each core outputs [1,2] raw partial sums; the host combines 8x2 scalars
    and applies the 1/5000 and 1/50000 normalizations.

kernel() takes the FULL (unsharded) inputs and returns the full output tuple
(contrastive_loss_sum, match_loss_sum, nonmatch_loss_sum) like the reference.
"""

import os

import numpy as np

# Problem constants (hardcoded per task spec).
B, N, D = 4, 307200, 16
M_MATCH, M_NONMATCH = 5000, 50000
MARGIN = 0.5
NON_MATCH_WEIGHT = 1.0
NCORES = 8

P = 128
MH = M_MATCH // 2  # 2500 match samples per core
NH = M_NONMATCH // 2  # 25000 nonmatch samples per core
MBLK = (MH + P - 1) // P  # 20 match blocks (last one partial: 2500=19*128+68)
NBLK = (NH + P - 1) // P  # 196 nonmatch blocks (25000=195*128+40)
M_REM = MH - (MBLK - 1) * P  # 68 real rows in last match block
N_REM = NH - (NBLK - 1) * P  # 40 real rows in last nonmatch block
CBLK = 28  # nonmatch blocks per compute chunk
NCH = NBLK // CBLK  # 7 chunks
assert NCH * CBLK == NBLK

LAST_EXEC_NS = None

_CACHE = {}


def _build_nc():
    import concourse.bacc as bacc
    import concourse.mybir as mybir
    import concourse.tile as tile
    from concourse import bass

    f32 = mybir.dt.float32
    i32 = mybir.dt.int32
    X = mybir.AxisListType.X
    ADD = mybir.AluOpType.add
    MULT = mybir.AluOpType.mult
    Relu = mybir.ActivationFunctionType.Relu

    nc = bacc.Bacc(
        "TRN2", target_bir_lowering=False, debug=False, num_swdge_queues=2
    )
    eA = nc.dram_tensor("eA", (N, D), f32, kind="ExternalInput")
    eB = nc.dram_tensor("eB", (N, D), f32, kind="ExternalInput")
    imA = nc.dram_tensor("imA", (P, MBLK), i32, kind="ExternalInput")
    imB = nc.dram_tensor("imB", (P, MBLK), i32, kind="ExternalInput")
    inA = nc.dram_tensor("inA", (P, NBLK), i32, kind="ExternalInput")
    inB = nc.dram_tensor("inB", (P, NBLK), i32, kind="ExternalInput")
    # pad handling: mmask is 1.0 for real match samples else 0.0;
    # npad adds 1e9 to pad nonmatch distances (hinge -> exactly 0)
    mmask = nc.dram_tensor("mmask", (P, MBLK), f32, kind="ExternalInput")
    npad = nc.dram_tensor("npad", (P, CBLK), f32, kind="ExternalInput")
    out = nc.dram_tensor("out", (1, 2), f32, kind="ExternalOutput")

    qctr = [0]

    def gather(dst_ap, src, idx_ap):
        inst = nc.gpsimd.indirect_dma_start(
            out=dst_ap,
            out_offset=None,
            in_=src.ap(),
            in_offset=bass.IndirectOffsetOnAxis(ap=idx_ap, axis=0),
        )
        # alternate the two SWDGE rings to overlap ring/doorbell bookkeeping
        if qctr[0] % 2:
            inst.ins.queue = "qPoolDynamic1"
        qctr[0] += 1

    with tile.TileContext(nc) as tc:
        with (
            tc.tile_pool(name="idx", bufs=1) as idxp,
            tc.tile_pool(name="gath", bufs=4) as gp,
            tc.tile_pool(name="cmp", bufs=4) as cp,
            tc.tile_pool(name="sums", bufs=1) as sp,
            tc.tile_pool(name="psum", bufs=1, space="PSUM") as pp,
        ):
            # index tiles (HWDGE loads; keep Pool free for the gather stream).
            # The first chunk's columns load first so gather 0 isn't gated on
            # the full index transfer.
            inA_t = idxp.tile([P, NBLK], i32)
            nc.sync.dma_start(inA_t[:, :CBLK], inA.ap()[:, :CBLK])
            inB_t = idxp.tile([P, NBLK], i32)
            nc.sync.dma_start(inB_t[:, :CBLK], inB.ap()[:, :CBLK])
            nc.sync.dma_start(inA_t[:, CBLK:], inA.ap()[:, CBLK:])
            nc.sync.dma_start(inB_t[:, CBLK:], inB.ap()[:, CBLK:])
            imA_t = idxp.tile([P, MBLK], i32)
            nc.sync.dma_start(imA_t[:], imA.ap())
            imB_t = idxp.tile([P, MBLK], i32)
            nc.sync.dma_start(imB_t[:], imB.ap())
            mmask_t = idxp.tile([P, MBLK], f32)
            nc.sync.dma_start(mmask_t[:], mmask.ap())
            npad_t = idxp.tile([P, CBLK], f32)
            nc.sync.dma_start(npad_t[:], npad.ap())

            sums = sp.tile([P, 1 + NCH], f32)
            margin_t = sp.tile([P, 1], f32)
            nc.vector.memset(margin_t[:], MARGIN)

            # --- nonmatch: gather + compute in chunks of CBLK blocks ---
            for c in range(NCH):
                ga = gp.tile([P, CBLK * D], f32, tag="ga")
                gb = gp.tile([P, CBLK * D], f32, tag="gb")
                for j in range(CBLK):
                    gj = c * CBLK + j
                    gather(ga[:, j * D : (j + 1) * D], eA, inA_t[:, gj : gj + 1])
                    gather(gb[:, j * D : (j + 1) * D], eB, inB_t[:, gj : gj + 1])

                nd = cp.tile([P, CBLK * D], f32, tag="nd")
                nc.vector.tensor_sub(nd[:], ga[:], gb[:])
                nsq = cp.tile([P, CBLK * D], f32, tag="nsq")
                nc.scalar.square(nsq[:], nd[:])
                dist = cp.tile([P, CBLK], f32, tag="dist")
                nc.vector.tensor_reduce(
                    dist[:],
                    nsq[:].rearrange("p (s d) -> p s d", d=D),
                    axis=X,
                    op=ADD,
                )
                if c == NCH - 1:
                    # pad samples: add 1e9 to their distance so the hinge
                    # is exactly 0
                    nc.vector.tensor_add(dist[:], dist[:], npad_t[:])
                hng = cp.tile([P, CBLK], f32, tag="hng")
                nc.scalar.activation(
                    hng[:],
                    dist[:],
                    Relu,
                    bias=margin_t[:],
                    scale=-1.0,
                    accum_out=sums[:, 1 + c : 2 + c],
                )

            # --- match: 20 blocks in one shot ---
            ma = gp.tile([P, MBLK * D], f32, tag="ma")
            mb = gp.tile([P, MBLK * D], f32, tag="mb")
            for j in range(MBLK):
                gather(ma[:, j * D : (j + 1) * D], eA, imA_t[:, j : j + 1])
                gather(mb[:, j * D : (j + 1) * D], eB, imB_t[:, j : j + 1])
            md = cp.tile([P, MBLK * D], f32, tag="md")
            nc.vector.tensor_sub(md[:], ma[:], mb[:])
            # mask the pad samples exactly: mdm = md * mmask (broadcast over D)
            mdm = cp.tile([P, MBLK * D], f32, tag="mdm")
            nc.vector.tensor_tensor(
                out=mdm[:].rearrange("p (s d) -> p s d", d=D),
                in0=md[:].rearrange("p (s d) -> p s d", d=D),
                in1=mmask_t[:].unsqueeze(2).to_broadcast([P, MBLK, D]),
                op=MULT,
            )
            msq = cp.tile([P, MBLK * D], f32, tag="msq")
            nc.scalar.activation(
                msq[:],
                mdm[:],
                mybir.ActivationFunctionType.Square,
                accum_out=sums[:, 0:1],
            )

            # --- cross-partition reduction: ones[128,1].T @ sums[128,1+NCH] ---
            ones = sp.tile([P, 1], f32)
            nc.vector.memset(ones[:], 1.0)
            acc = pp.tile([1, 1 + NCH], f32, space="PSUM")
            nc.tensor.matmul(acc[:], lhsT=ones[:], rhs=sums[:], start=True, stop=True)
            res = sp.tile([1, 2], f32)
            nc.vector.tensor_copy(res[:, 0:1], acc[:, 0:1])
            nc.vector.tensor_reduce(res[:, 1:2], acc[:, 1 : 1 + NCH], axis=X, op=ADD)
            nc.sync.dma_start(out.ap(), res[:])

    nc.compile()
    return nc


def _get_nc():
    if "nc" not in _CACHE:
        _CACHE["nc"] = _build_nc()
    return _CACHE["nc"]


def _blocked(idx_1d, nblocks):
    """[n] -> [128, nblocks] with sample s at [s % 128, s // 128]; pad with 0."""
    out = np.zeros((P, nblocks), np.int32)
    n = idx_1d.shape[0]
    full = n // P
    out[:, :full] = idx_1d[: full * P].reshape(full, P).T
    rem = n - full * P
    if rem:
        out[:rem, full] = idx_1d[full * P :]
    return out


def _in_maps(outA, outB, matchA, matchB, nonMatchA, nonMatchB):
    outA = np.ascontiguousarray(np.asarray(outA, dtype=np.float32))
    outB = np.ascontiguousarray(np.asarray(outB, dtype=np.float32))
    matchA = np.asarray(matchA).astype(np.int32)
    matchB = np.asarray(matchB).astype(np.int32)
    nonMatchA = np.asarray(nonMatchA).astype(np.int32)
    nonMatchB = np.asarray(nonMatchB).astype(np.int32)

    mmask = np.zeros((P, MBLK), np.float32)
    mmask[:, : MBLK - 1] = 1.0
    mmask[:M_REM, MBLK - 1] = 1.0
    npad = np.zeros((P, CBLK), np.float32)
    npad[N_REM:, CBLK - 1] = 1e9

    maps = []
    for c in range(NCORES):
        b, h = c // 2, c % 2
        maps.append(
            {
                "eA": outA[b],
                "eB": outB[b],
                "imA": _blocked(matchA[b, h * MH : (h + 1) * MH], MBLK),
                "imB": _blocked(matchB[b, h * MH : (h + 1) * MH], MBLK),
                "inA": _blocked(nonMatchA[b, h * NH : (h + 1) * NH], NBLK),
                "inB": _blocked(nonMatchB[b, h * NH : (h + 1) * NH], NBLK),
                "mmask": mmask,
                "npad": npad,
            }
        )
    return maps


def kernel(outA, outB, matchA, matchB, nonMatchA, nonMatchB):
    global LAST_EXEC_NS
    from concourse import bass_utils

    nc = _get_nc()
    maps = _in_maps(outA, outB, matchA, matchB, nonMatchA, nonMatchB)

    kwargs = {}
    if os.environ.get("KERNEL_TRACE", "0") == "1":
        kwargs["trace"] = True
    r = bass_utils.run_bass_kernel_spmd(
        nc, maps, core_ids=list(range(NCORES)), **kwargs
    )
    LAST_EXEC_NS = r.exec_time_ns

    partial = np.stack(
        [np.asarray(r.results[c]["out"]).ravel() for c in range(NCORES)]
    )
    match_loss = partial[:, 0].sum(dtype=np.float64) / M_MATCH
    nonmatch_loss = (
        NON_MATCH_WEIGHT * partial[:, 1].sum(dtype=np.float64) / M_NONMATCH
    )
    contrastive = match_loss + nonmatch_loss
    return (
        np.float32(contrastive),
        np.float32(match_loss),
        np.float32(nonmatch_loss),
    )



# revision 12
# speedup vs baseline: 1.1792x; 1.1792x over previous
"""ContrastiveLoss Trainium2 kernel.

Strategy (data-parallel over 8 NeuronCores):
  - 8 cores = 4 batches x 2 halves. Core c handles batch b=c//2, half h=c%2:
    2500 match pairs + 25000 non-match pairs.
  - Gather primitive: SWDGE vector-indirect DMA (`indirect_dma_start`), which
    on TRN2 fetches ONE dynamic row index per SBUF partition and streams the
    row (16 f32 = 64 B) into that partition. One instruction = 128 rows.
    Samples are column-blocked: sample s = block j * 128 + partition p, so
    block j's indices live in idx[:, j] and its rows land in g[:, 16j:16j+16].
  - Per-sample math on DVE/ACT (fully hidden under the gather stream):
      match partial  = sum((mA-mB)^2)              (DVE sub + fused sq-reduce)
      nonmatch partial = sum(relu(0.5-||nA-nB||^2)) (DVE sub, ACT square,
        DVE grouped reduce over D=16, ACT relu with fused accumulate)
  - Padding: tail samples use index 0 on both sides; a host-built {0,1} mask
    zeroes pad match diffs exactly, and a host-built additive bias pushes pad
    nonmatch distances to 1e9 so the hinge is exactly 0.
  - Partition reduction via a ones-vector TensorE matmul into PSUM.
  - Each core outputs [1,2] raw partial sums; the host combines 8x2 scalars
    and applies the 1/5000 and 1/50000 normalizations.

kernel() takes the FULL (unsharded) inputs and returns the full output tuple
(contrastive_loss_sum, match_loss_sum, nonmatch_loss_sum) like the reference.
"""

import os

import numpy as np

# Problem constants (hardcoded per task spec).
B, N, D = 4, 307200, 16
M_MATCH, M_NONMATCH = 5000, 50000
MARGIN = 0.5
NON_MATCH_WEIGHT = 1.0
NCORES = 8

P = 128
MH = M_MATCH // 2  # 2500 match samples per core
NH = M_NONMATCH // 2  # 25000 nonmatch samples per core
MBLK = (MH + P - 1) // P  # 20 match blocks (last one partial: 2500=19*128+68)
NBLK = (NH + P - 1) // P  # 196 nonmatch blocks (25000=195*128+40)
M_REM = MH - (MBLK - 1) * P  # 68 real rows in last match block
N_REM = NH - (NBLK - 1) * P  # 40 real rows in last nonmatch block
CBLK = 28  # nonmatch blocks per compute chunk
NCH = NBLK // CBLK  # 7 chunks
assert NCH * CBLK == NBLK

LAST_EXEC_NS = None

_CACHE = {}


def _build_nc():
    import concourse.bacc as bacc
    import concourse.mybir as mybir
    import concourse.tile as tile
    from concourse import bass

    f32 = mybir.dt.float32
    i32 = mybir.dt.int32
    X = mybir.AxisListType.X
    ADD = mybir.AluOpType.add
    MULT = mybir.AluOpType.mult
    Relu = mybir.ActivationFunctionType.Relu

    nc = bacc.Bacc(
        "TRN2", target_bir_lowering=False, debug=False, num_swdge_queues=2
    )
    eA = nc.dram_tensor("eA", (N, D), f32, kind="ExternalInput")
    eB = nc.dram_tensor("eB", (N, D), f32, kind="ExternalInput")
    imA = nc.dram_tensor("imA", (P, MBLK), i32, kind="ExternalInput")
    imB = nc.dram_tensor("imB", (P, MBLK), i32, kind="ExternalInput")
    inA = nc.dram_tensor("inA", (P, NBLK), i32, kind="ExternalInput")
    inB = nc.dram_tensor("inB", (P, NBLK), i32, kind="ExternalInput")
    # pad handling: mmask is 1.0 for real match samples else 0.0;
    # npad adds 1e9 to pad nonmatch distances (hinge -> exactly 0)
    mmask = nc.dram_tensor("mmask", (P, MBLK), f32, kind="ExternalInput")
    npad = nc.dram_tensor("npad", (P, CBLK), f32, kind="ExternalInput")
    out = nc.dram_tensor("out", (1, 2), f32, kind="ExternalOutput")

    qctr = [0]

    def gather(dst_ap, src, idx_ap):
        inst = nc.gpsimd.indirect_dma_start(
            out=dst_ap,
            out_offset=None,
            in_=src.ap(),
            in_offset=bass.IndirectOffsetOnAxis(ap=idx_ap, axis=0),
        )
        # alternate the two SWDGE rings to overlap ring/doorbell bookkeeping
        if qctr[0] % 2:
            inst.ins.queue = "qPoolDynamic1"
        qctr[0] += 1

    with tile.TileContext(nc) as tc:
        with (
            tc.tile_pool(name="idx", bufs=1) as idxp,
            tc.tile_pool(name="gath", bufs=3) as gp,
            tc.tile_pool(name="cmp", bufs=3) as cp,
            tc.tile_pool(name="sums", bufs=1) as sp,
            tc.tile_pool(name="psum", bufs=1, space="PSUM") as pp,
        ):
            # index tiles (HWDGE loads; keep Pool free for the gather stream)
            inA_t = idxp.tile([P, NBLK], i32)
            nc.sync.dma_start(inA_t[:], inA.ap())
            inB_t = idxp.tile([P, NBLK], i32)
            nc.sync.dma_start(inB_t[:], inB.ap())
            imA_t = idxp.tile([P, MBLK], i32)
            nc.sync.dma_start(imA_t[:], imA.ap())
            imB_t = idxp.tile([P, MBLK], i32)
            nc.sync.dma_start(imB_t[:], imB.ap())
            mmask_t = idxp.tile([P, MBLK], f32)
            nc.sync.dma_start(mmask_t[:], mmask.ap())
            npad_t = idxp.tile([P, CBLK], f32)
            nc.sync.dma_start(npad_t[:], npad.ap())

            sums = sp.tile([P, 1 + NCH], f32)
            margin_t = sp.tile([P, 1], f32)
            nc.vector.memset(margin_t[:], MARGIN)

            # --- nonmatch: gather + compute in chunks of CBLK blocks ---
            for c in range(NCH):
                ga = gp.tile([P, CBLK * D], f32, tag="ga")
                gb = gp.tile([P, CBLK * D], f32, tag="gb")
                for j in range(CBLK):
                    gj = c * CBLK + j
                    gather(ga[:, j * D : (j + 1) * D], eA, inA_t[:, gj : gj + 1])
                    gather(gb[:, j * D : (j + 1) * D], eB, inB_t[:, gj : gj + 1])

                nd = cp.tile([P, CBLK * D], f32, tag="nd")
                nc.vector.tensor_sub(nd[:], ga[:], gb[:])
                nsq = cp.tile([P, CBLK * D], f32, tag="nsq")
                nc.scalar.square(nsq[:], nd[:])
                dist = cp.tile([P, CBLK], f32, tag="dist")
                nc.vector.tensor_reduce(
                    dist[:],
                    nsq[:].rearrange("p (s d) -> p s d", d=D),
                    axis=X,
                    op=ADD,
                )
                if c == NCH - 1:
                    # pad samples: add 1e9 to their distance so the hinge
                    # is exactly 0
                    nc.vector.tensor_add(dist[:], dist[:], npad_t[:])
                hng = cp.tile([P, CBLK], f32, tag="hng")
                nc.scalar.activation(
                    hng[:],
                    dist[:],
                    Relu,
                    bias=margin_t[:],
                    scale=-1.0,
                    accum_out=sums[:, 1 + c : 2 + c],
                )

            # --- match: 20 blocks in one shot ---
            ma = gp.tile([P, MBLK * D], f32, tag="ma")
            mb = gp.tile([P, MBLK * D], f32, tag="mb")
            for j in range(MBLK):
                gather(ma[:, j * D : (j + 1) * D], eA, imA_t[:, j : j + 1])
                gather(mb[:, j * D : (j + 1) * D], eB, imB_t[:, j : j + 1])
            md = cp.tile([P, MBLK * D], f32, tag="md")
            nc.vector.tensor_sub(md[:], ma[:], mb[:])
            # mask the pad samples exactly: mdm = md * mmask (broadcast over D)
            mdm = cp.tile([P, MBLK * D], f32, tag="mdm")
            nc.vector.tensor_tensor(
                out=mdm[:].rearrange("p (s d) -> p s d", d=D),
                in0=md[:].rearrange("p (s d) -> p s d", d=D),
                in1=mmask_t[:].unsqueeze(2).to_broadcast([P, MBLK, D]),
                op=MULT,
            )
            msq = cp.tile([P, MBLK * D], f32, tag="msq")
            nc.scalar.activation(
                msq[:],
                mdm[:],
                mybir.ActivationFunctionType.Square,
                accum_out=sums[:, 0:1],
            )

            # --- cross-partition reduction: ones[128,1].T @ sums[128,1+NCH] ---
            ones = sp.tile([P, 1], f32)
            nc.vector.memset(ones[:], 1.0)
            acc = pp.tile([1, 1 + NCH], f32, space="PSUM")
            nc.tensor.matmul(acc[:], lhsT=ones[:], rhs=sums[:], start=True, stop=True)
            res = sp.tile([1, 2], f32)
            nc.vector.tensor_copy(res[:, 0:1], acc[:, 0:1])
            nc.vector.tensor_reduce(res[:, 1:2], acc[:, 1 : 1 + NCH], axis=X, op=ADD)
            nc.sync.dma_start(out.ap(), res[:])

    nc.compile()
    return nc


def _get_nc():
    if "nc" not in _CACHE:
        _CACHE["nc"] = _build_nc()
    return _CACHE["nc"]


def _blocked(idx_1d, nblocks):
    """[n] -> [128, nblocks] with sample s at [s % 128, s // 128]; pad with 0."""
    out = np.zeros((P, nblocks), np.int32)
    n = idx_1d.shape[0]
    full = n // P
    out[:, :full] = idx_1d[: full * P].reshape(full, P).T
    rem = n - full * P
    if rem:
        out[:rem, full] = idx_1d[full * P :]
    return out


def _in_maps(outA, outB, matchA, matchB, nonMatchA, nonMatchB):
    outA = np.ascontiguousarray(np.asarray(outA, dtype=np.float32))
    outB = np.ascontiguousarray(np.asarray(outB, dtype=np.float32))
    matchA = np.asarray(matchA).astype(np.int32)
    matchB = np.asarray(matchB).astype(np.int32)
    nonMatchA = np.asarray(nonMatchA).astype(np.int32)
    nonMatchB = np.asarray(nonMatchB).astype(np.int32)

    mmask = np.zeros((P, MBLK), np.float32)
    mmask[:, : MBLK - 1] = 1.0
    mmask[:M_REM, MBLK - 1] = 1.0
    npad = np.zeros((P, CBLK), np.float32)
    npad[N_REM:, CBLK - 1] = 1e9

    maps = []
    for c in range(NCORES):
        b, h = c // 2, c % 2
        maps.append(
            {
                "eA": outA[b],
                "eB": outB[b],
                "imA": _blocked(matchA[b, h * MH : (h + 1) * MH], MBLK),
                "imB": _blocked(matchB[b, h * MH : (h + 1) * MH], MBLK),
                "inA": _blocked(nonMatchA[b, h * NH : (h + 1) * NH], NBLK),
                "inB": _blocked(nonMatchB[b, h * NH : (h + 1) * NH], NBLK),
                "mmask": mmask,
                "npad": npad,
            }
        )
    return maps


def kernel(outA, outB, matchA, matchB, nonMatchA, nonMatchB):
    global LAST_EXEC_NS
    from concourse import bass_utils

    nc = _get_nc()
    maps = _in_maps(outA, outB, matchA, matchB, nonMatchA, nonMatchB)

    kwargs = {}
    if os.environ.get("KERNEL_TRACE", "0") == "1":
        kwargs["trace"] = True
    r = bass_utils.run_bass_kernel_spmd(
        nc, maps, core_ids=list(range(NCORES)), **kwargs
    )
    LAST_EXEC_NS = r.exec_time_ns

    partial = np.stack(
        [np.asarray(r.results[c]["out"]).ravel() for c in range(NCORES)]
    )
    match_loss = partial[:, 0].sum(dtype=np.float64) / M_MATCH
    nonmatch_loss = (
        NON_MATCH_WEIGHT * partial[:, 1].sum(dtype=np.float64) / M_NONMATCH
    )
    contrastive = match_loss + nonmatch_loss
    return (
        np.float32(contrastive),
        np.float32(match_loss),
        np.float32(nonmatch_loss),
    )



# revision 13
# speedup vs baseline: 1.2567x; 1.0657x over previous
"""ContrastiveLoss Trainium2 kernel.

Strategy (data-parallel over 8 NeuronCores):
  - 8 cores = 4 batches x 2 halves. Core c handles batch b=c//2, half h=c%2:
    2500 match pairs + 25000 non-match pairs.
  - Gather primitive: SWDGE vector-indirect DMA (`indirect_dma_start`), which
    on TRN2 fetches ONE dynamic row index per SBUF partition and streams the
    row (16 f32 = 64 B) into that partition. One instruction = 128 rows.
    Samples are column-blocked: sample s = block j * 128 + partition p, so
    block j's indices live in idx[:, j] and its rows land in g[:, 16j:16j+16].
  - Per-sample math on DVE/ACT (fully hidden under the gather stream):
      match partial  = sum((mA-mB)^2)              (DVE sub + fused sq-reduce)
      nonmatch partial = sum(relu(0.5-||nA-nB||^2)) (DVE sub, ACT square,
        DVE grouped reduce over D=16, ACT relu with fused accumulate)
  - Padding: tail samples use index 0 on both sides; a host-built {0,1} mask
    zeroes pad match diffs exactly, and a host-built additive bias pushes pad
    nonmatch distances to 1e9 so the hinge is exactly 0.
  - Partition reduction via a ones-vector TensorE matmul into PSUM.
  - Each core outputs [1,2] raw partial sums; the host combines 8x2 scalars
    and applies the 1/5000 and 1/50000 normalizations.

kernel() takes the FULL (unsharded) inputs and returns the full output tuple
(contrastive_loss_sum, match_loss_sum, nonmatch_loss_sum) like the reference.
"""

import os

import numpy as np

# Problem constants (hardcoded per task spec).
B, N, D = 4, 307200, 16
M_MATCH, M_NONMATCH = 5000, 50000
MARGIN = 0.5
NON_MATCH_WEIGHT = 1.0
NCORES = 8

P = 128
MH = M_MATCH // 2  # 2500 match samples per core
NH = M_NONMATCH // 2  # 25000 nonmatch samples per core
MBLK = (MH + P - 1) // P  # 20 match blocks (last one partial: 2500=19*128+68)
NBLK = (NH + P - 1) // P  # 196 nonmatch blocks (25000=195*128+40)
M_REM = MH - (MBLK - 1) * P  # 68 real rows in last match block
N_REM = NH - (NBLK - 1) * P  # 40 real rows in last nonmatch block
CBLK = 28  # nonmatch blocks per compute chunk
NCH = NBLK // CBLK  # 7 chunks
assert NCH * CBLK == NBLK

LAST_EXEC_NS = None

_CACHE = {}


def _build_nc():
    import concourse.bacc as bacc
    import concourse.mybir as mybir
    import concourse.tile as tile
    from concourse import bass

    f32 = mybir.dt.float32
    i32 = mybir.dt.int32
    X = mybir.AxisListType.X
    ADD = mybir.AluOpType.add
    MULT = mybir.AluOpType.mult
    Relu = mybir.ActivationFunctionType.Relu

    nc = bacc.Bacc(
        "TRN2", target_bir_lowering=False, debug=False, num_swdge_queues=2
    )
    eA = nc.dram_tensor("eA", (N, D), f32, kind="ExternalInput")
    eB = nc.dram_tensor("eB", (N, D), f32, kind="ExternalInput")
    imA = nc.dram_tensor("imA", (P, MBLK), i32, kind="ExternalInput")
    imB = nc.dram_tensor("imB", (P, MBLK), i32, kind="ExternalInput")
    inA = nc.dram_tensor("inA", (P, NBLK), i32, kind="ExternalInput")
    inB = nc.dram_tensor("inB", (P, NBLK), i32, kind="ExternalInput")
    # pad handling: mmask is 1.0 for real match samples else 0.0;
    # npad adds 1e9 to pad nonmatch distances (hinge -> exactly 0)
    mmask = nc.dram_tensor("mmask", (P, MBLK), f32, kind="ExternalInput")
    npad = nc.dram_tensor("npad", (P, CBLK), f32, kind="ExternalInput")
    out = nc.dram_tensor("out", (1, 2), f32, kind="ExternalOutput")

    qctr = [0]

    def gather(dst_ap, src, idx_ap):
        inst = nc.gpsimd.indirect_dma_start(
            out=dst_ap,
            out_offset=None,
            in_=src.ap(),
            in_offset=bass.IndirectOffsetOnAxis(ap=idx_ap, axis=0),
        )
        # alternate the two SWDGE rings to overlap ring/doorbell bookkeeping
        if qctr[0] % 2:
            inst.ins.queue = "qPoolDynamic1"
        qctr[0] += 1

    with tile.TileContext(nc) as tc:
        with (
            tc.tile_pool(name="idx", bufs=1) as idxp,
            tc.tile_pool(name="gath", bufs=4) as gp,
            tc.tile_pool(name="cmp", bufs=4) as cp,
            tc.tile_pool(name="sums", bufs=1) as sp,
            tc.tile_pool(name="psum", bufs=1, space="PSUM") as pp,
        ):
            # index tiles (HWDGE loads; keep Pool free for the gather stream)
            inA_t = idxp.tile([P, NBLK], i32)
            nc.sync.dma_start(inA_t[:, :CBLK], inA.ap()[:, :CBLK])
            inB_t = idxp.tile([P, NBLK], i32)
            nc.sync.dma_start(inB_t[:, :CBLK], inB.ap()[:, :CBLK])
            nc.sync.dma_start(inA_t[:, CBLK:], inA.ap()[:, CBLK:])
            nc.sync.dma_start(inB_t[:, CBLK:], inB.ap()[:, CBLK:])
            imA_t = idxp.tile([P, MBLK], i32)
            nc.sync.dma_start(imA_t[:], imA.ap())
            imB_t = idxp.tile([P, MBLK], i32)
            nc.sync.dma_start(imB_t[:], imB.ap())
            mmask_t = idxp.tile([P, MBLK], f32)
            nc.sync.dma_start(mmask_t[:], mmask.ap())
            npad_t = idxp.tile([P, CBLK], f32)
            nc.sync.dma_start(npad_t[:], npad.ap())

            sums = sp.tile([P, 1 + NCH], f32)
            margin_t = sp.tile([P, 1], f32)
            nc.vector.memset(margin_t[:], MARGIN)

            # --- nonmatch: gather + compute in chunks of CBLK blocks ---
            for c in range(NCH):
                ga = gp.tile([P, CBLK * D], f32, tag="ga")
                gb = gp.tile([P, CBLK * D], f32, tag="gb")
                for j in range(CBLK):
                    gj = c * CBLK + j
                    gather(ga[:, j * D : (j + 1) * D], eA, inA_t[:, gj : gj + 1])
                    gather(gb[:, j * D : (j + 1) * D], eB, inB_t[:, gj : gj + 1])

                nd = cp.tile([P, CBLK * D], f32, tag="nd")
                nc.vector.tensor_sub(nd[:], ga[:], gb[:])
                nsq = cp.tile([P, CBLK * D], f32, tag="nsq")
                nc.scalar.square(nsq[:], nd[:])
                dist = cp.tile([P, CBLK], f32, tag="dist")
                nc.vector.tensor_reduce(
                    dist[:],
                    nsq[:].rearrange("p (s d) -> p s d", d=D),
                    axis=X,
                    op=ADD,
                )
                if c == NCH - 1:
                    # pad samples: add 1e9 to their distance so the hinge
                    # is exactly 0
                    nc.vector.tensor_add(dist[:], dist[:], npad_t[:])
                hng = cp.tile([P, CBLK], f32, tag="hng")
                nc.scalar.activation(
                    hng[:],
                    dist[:],
                    Relu,
                    bias=margin_t[:],
                    scale=-1.0,
                    accum_out=sums[:, 1 + c : 2 + c],
                )

            # --- match: 20 blocks in one shot ---
            ma = gp.tile([P, MBLK * D], f32, tag="ma")
            mb = gp.tile([P, MBLK * D], f32, tag="mb")
            for j in range(MBLK):
                gather(ma[:, j * D : (j + 1) * D], eA, imA_t[:, j : j + 1])
                gather(mb[:, j * D : (j + 1) * D], eB, imB_t[:, j : j + 1])
            md = cp.tile([P, MBLK * D], f32, tag="md")
            nc.vector.tensor_sub(md[:], ma[:], mb[:])
            # mask the pad samples exactly: mdm = md * mmask (broadcast over D)
            mdm = cp.tile([P, MBLK * D], f32, tag="mdm")
            nc.vector.tensor_tensor(
                out=mdm[:].rearrange("p (s d) -> p s d", d=D),
                in0=md[:].rearrange("p (s d) -> p s d", d=D),
                in1=mmask_t[:].unsqueeze(2).to_broadcast([P, MBLK, D]),
                op=MULT,
            )
            msq = cp.tile([P, MBLK * D], f32, tag="msq")
            nc.scalar.activation(
                msq[:],
                mdm[:],
                mybir.ActivationFunctionType.Square,
                accum_out=sums[:, 0:1],
            )

            # --- cross-partition reduction: ones[128,1].T @ sums[128,1+NCH] ---
            ones = sp.tile([P, 1], f32)
            nc.vector.memset(ones[:], 1.0)
            acc = pp.tile([1, 1 + NCH], f32, space="PSUM")
            nc.tensor.matmul(acc[:], lhsT=ones[:], rhs=sums[:], start=True, stop=True)
            res = sp.tile([1, 2], f32)
            nc.vector.tensor_copy(res[:, 0:1], acc[:, 0:1])
            nc.vector.tensor_reduce(res[:, 1:2], acc[:, 1 : 1 + NCH], axis=X, op=ADD)
            nc.sync.dma_start(out.ap(), res[:])

    nc.compile()
    return nc


def _get_nc():
    if "nc" not in _CACHE:
        _CACHE["nc"] = _build_nc()
    return _CACHE["nc"]


def _blocked(idx_1d, nblocks):
    """[n] -> [128, nblocks] with sample s at [s % 128, s // 128]; pad with 0."""
    out = np.zeros((P, nblocks), np.int32)
    n = idx_1d.shape[0]
    full = n // P
    out[:, :full] = idx_1d[: full * P].reshape(full, P).T
    rem = n - full * P
    if rem:
        out[:rem, full] = idx_1d[full * P :]
    return out


def _in_maps(outA, outB, matchA, matchB, nonMatchA, nonMatchB):
    outA = np.ascontiguousarray(np.asarray(outA, dtype=np.float32))
    outB = np.ascontiguousarray(np.asarray(outB, dtype=np.float32))
    matchA = np.asarray(matchA).astype(np.int32)
    matchB = np.asarray(matchB).astype(np.int32)
    nonMatchA = np.asarray(nonMatchA).astype(np.int32)
    nonMatchB = np.asarray(nonMatchB).astype(np.int32)

    mmask = np.zeros((P, MBLK), np.float32)
    mmask[:, : MBLK - 1] = 1.0
    mmask[:M_REM, MBLK - 1] = 1.0
    npad = np.zeros((P, CBLK), np.float32)
    npad[N_REM:, CBLK - 1] = 1e9

    maps = []
    for c in range(NCORES):
        b, h = c // 2, c % 2
        maps.append(
            {
                "eA": outA[b],
                "eB": outB[b],
                "imA": _blocked(matchA[b, h * MH : (h + 1) * MH], MBLK),
                "imB": _blocked(matchB[b, h * MH : (h + 1) * MH], MBLK),
                "inA": _blocked(nonMatchA[b, h * NH : (h + 1) * NH], NBLK),
                "inB": _blocked(nonMatchB[b, h * NH : (h + 1) * NH], NBLK),
                "mmask": mmask,
                "npad": npad,
            }
        )
    return maps


def kernel(outA, outB, matchA, matchB, nonMatchA, nonMatchB):
    global LAST_EXEC_NS
    from concourse import bass_utils

    nc = _get_nc()
    maps = _in_maps(outA, outB, matchA, matchB, nonMatchA, nonMatchB)

    kwargs = {}
    if os.environ.get("KERNEL_TRACE", "0") == "1":
        kwargs["trace"] = True
    r = bass_utils.run_bass_kernel_spmd(
        nc, maps, core_ids=list(range(NCORES)), **kwargs
    )
    LAST_EXEC_NS = r.exec_time_ns

    partial = np.stack(
        [np.asarray(r.results[c]["out"]).ravel() for c in range(NCORES)]
    )
    match_loss = partial[:, 0].sum(dtype=np.float64) / M_MATCH
    nonmatch_loss = (
        NON_MATCH_WEIGHT * partial[:, 1].sum(dtype=np.float64) / M_NONMATCH
    )
    contrastive = match_loss + nonmatch_loss
    return (
        np.float32(contrastive),
        np.float32(match_loss),
        np.float32(nonmatch_loss),
    )



# revision 14
# speedup vs baseline: 1.2585x; 1.0014x over previous
"""ContrastiveLoss Trainium2 kernel — adjacency-paired gathers.

Same data-parallel layout as kernel.py (8 cores = 4 batches x 2 halves,
27500 sample pairs per core), same indirect1d gather primitive (one index
per SBUF partition per instruction, streaming the out free dim contiguously
from that row — hardware-verified semantics).

New: the per-instruction cost (~1.4 us) is independent of the streamed
length, so samples whose A-rows (or B-rows) are CONSECUTIVE in DRAM are
host-paired and fetched by one C=2 descriptor streaming 2 rows (128 B) into
two adjacent slot columns. Greedy pairing on the A side, then on the B side
among the rest, removes (pA+pB)/128 ~ 25 of the 432 gather instructions.

Slot layout per core (COLS columns x 128 partitions):
  [A-pair region: 2*nPA cols][B-pair region: 2*nPB cols][singles + pads]
A-pair instruction k reads ia[:, 2k] and writes columns 2k, 2k+1 (the host
guarantees ia[p, 2k+1] == ia[p, 2k]+1); its B side uses normal C=1 columns.
Per-slot f32 weights wm/wn (1.0 for match/nonmatch, 0 for pads) replace the
old block masks: dist -> match partial = sum dist*wm, nonmatch partial =
sum relu(0.5-dist)*wn, partition-reduced by a ones-vector matmul.
"""

import os

import numpy as np

B, N, D = 4, 307200, 16
M_MATCH, M_NONMATCH = 5000, 50000
MARGIN = 0.5
NON_MATCH_WEIGHT = 1.0
NCORES = 8

P = 128
MH = M_MATCH // 2
NH = M_NONMATCH // 2
NS = MH + NH  # 27500 samples per core
CHUNK = 32  # compute-chunk width in columns (even, so pairs never straddle)

LAST_EXEC_NS = None
_CACHE = {}


def _pair_scan(vals, order):
    """Greedy adjacent-value pairing over `order` (indices into vals, sorted
    by vals). Returns list of (lower_sample, upper_sample)."""
    pairs = []
    i = 0
    while i < len(order) - 1:
        s0, s1 = order[i], order[i + 1]
        if vals[s1] == vals[s0] + 1:
            pairs.append((s0, s1))
            i += 2
        else:
            i += 1
    return pairs


def _plan(a, b):
    """Pair samples on A, then on B among the rest. Returns (pairsA, pairsB,
    leftover order)."""
    n = len(a)
    orderA = np.argsort(a, kind="stable")
    pairsA = _pair_scan(a, orderA)
    used = np.zeros(n, np.bool_)
    for s0, s1 in pairsA:
        used[s0] = used[s1] = True
    rem = np.where(~used)[0]
    orderB = rem[np.argsort(b[rem], kind="stable")]
    pairsB = _pair_scan(b, orderB)
    for s0, s1 in pairsB:
        used[s0] = used[s1] = True
    return pairsA, pairsB


def _build_nc(nPA, nPB, COLS):
    import concourse.bacc as bacc
    import concourse.mybir as mybir
    import concourse.tile as tile
    from concourse import bass

    f32 = mybir.dt.float32
    i32 = mybir.dt.int32
    X = mybir.AxisListType.X
    ADD = mybir.AluOpType.add
    MULT = mybir.AluOpType.mult
    Relu = mybir.ActivationFunctionType.Relu

    nc = bacc.Bacc(
        "TRN2", target_bir_lowering=False, debug=False, num_swdge_queues=2
    )
    eA = nc.dram_tensor("eA", (N, D), f32, kind="ExternalInput")
    eB = nc.dram_tensor("eB", (N, D), f32, kind="ExternalInput")
    ia = nc.dram_tensor("ia", (P, COLS), i32, kind="ExternalInput")
    ib = nc.dram_tensor("ib", (P, COLS), i32, kind="ExternalInput")
    wm = nc.dram_tensor("wm", (P, COLS), f32, kind="ExternalInput")
    wn = nc.dram_tensor("wn", (P, COLS), f32, kind="ExternalInput")
    out = nc.dram_tensor("out", (1, 2), f32, kind="ExternalOutput")

    qctr = [0]

    def gather(dst_ap, src, idx_ap):
        inst = nc.gpsimd.indirect_dma_start(
            out=dst_ap,
            out_offset=None,
            in_=src.ap(),
            in_offset=bass.IndirectOffsetOnAxis(ap=idx_ap, axis=0),
        )
        if qctr[0] % 2:
            inst.ins.queue = "qPoolDynamic1"
        qctr[0] += 1

    with tile.TileContext(nc) as tc:
        with (
            tc.tile_pool(name="io", bufs=1) as iop,
            tc.tile_pool(name="gath", bufs=1) as gp,
            tc.tile_pool(name="cmp", bufs=4) as cp,
            tc.tile_pool(name="psum", bufs=1, space="PSUM") as pp,
        ):
            # first compute chunk's indices load first
            c0 = min(CHUNK, COLS)
            ia_t = iop.tile([P, COLS], i32)
            nc.sync.dma_start(ia_t[:, :c0], ia.ap()[:, :c0])
            ib_t = iop.tile([P, COLS], i32)
            nc.sync.dma_start(ib_t[:, :c0], ib.ap()[:, :c0])
            if COLS > c0:
                nc.sync.dma_start(ia_t[:, c0:], ia.ap()[:, c0:])
                nc.sync.dma_start(ib_t[:, c0:], ib.ap()[:, c0:])
            wm_t = iop.tile([P, COLS], f32)
            nc.sync.dma_start(wm_t[:], wm.ap())
            wn_t = iop.tile([P, COLS], f32)
            nc.sync.dma_start(wn_t[:], wn.ap())
            margin_t = iop.tile([P, 1], f32)
            nc.vector.memset(margin_t[:], MARGIN)

            gA = gp.tile([P, COLS * D], f32)
            gB = gp.tile([P, COLS * D], f32)
            dist = gp.tile([P, COLS], f32)
            hng = gp.tile([P, COLS], f32)

            pb_base = 2 * nPA
            s_base = 2 * nPA + 2 * nPB

            def emit_col(c):
                if c < pb_base:
                    if c % 2 == 0:  # A-pair: one C=2 descriptor, 2 columns
                        gather(gA[:, c * D : (c + 2) * D], eA, ia_t[:, c : c + 1])
                    gather(gB[:, c * D : (c + 1) * D], eB, ib_t[:, c : c + 1])
                elif c < s_base:
                    if (c - pb_base) % 2 == 0:  # B-pair
                        gather(gB[:, c * D : (c + 2) * D], eB, ib_t[:, c : c + 1])
                    gather(gA[:, c * D : (c + 1) * D], eA, ia_t[:, c : c + 1])
                else:
                    gather(gA[:, c * D : (c + 1) * D], eA, ia_t[:, c : c + 1])
                    gather(gB[:, c * D : (c + 1) * D], eB, ib_t[:, c : c + 1])

            for cs in range(0, COLS, CHUNK):
                ce = min(cs + CHUNK, COLS)
                for c in range(cs, ce):
                    emit_col(c)
                w = ce - cs
                nd = cp.tile([P, CHUNK * D], f32, tag="nd")
                nc.vector.tensor_sub(
                    nd[:, : w * D], gA[:, cs * D : ce * D], gB[:, cs * D : ce * D]
                )
                nsq = cp.tile([P, CHUNK * D], f32, tag="nsq")
                nc.scalar.square(nsq[:, : w * D], nd[:, : w * D])
                nc.vector.tensor_reduce(
                    dist[:, cs:ce],
                    nsq[:, : w * D].rearrange("p (s d) -> p s d", d=D),
                    axis=X,
                    op=ADD,
                )
                nc.scalar.activation(
                    hng[:, cs:ce],
                    dist[:, cs:ce],
                    Relu,
                    bias=margin_t[:],
                    scale=-1.0,
                )

            # weighted partials + cross-partition reduction
            sums = gp.tile([P, 2], f32)
            md = cp.tile([P, COLS], f32, tag="md")
            nc.vector.tensor_tensor(out=md[:], in0=dist[:], in1=wm_t[:], op=MULT)
            nc.vector.tensor_reduce(sums[:, 0:1], md[:], axis=X, op=ADD)
            nh = cp.tile([P, COLS], f32, tag="nh")
            nc.vector.tensor_tensor(out=nh[:], in0=hng[:], in1=wn_t[:], op=MULT)
            nc.vector.tensor_reduce(sums[:, 1:2], nh[:], axis=X, op=ADD)

            ones = gp.tile([P, 1], f32)
            nc.vector.memset(ones[:], 1.0)
            acc = pp.tile([1, 2], f32, space="PSUM")
            nc.tensor.matmul(acc[:], lhsT=ones[:], rhs=sums[:], start=True, stop=True)
            res = gp.tile([1, 2], f32)
            nc.vector.tensor_copy(res[:], acc[:])
            nc.sync.dma_start(out.ap(), res[:])

    nc.compile()
    return nc


def _in_maps(outA, outB, matchA, matchB, nonMatchA, nonMatchB):
    outA = np.ascontiguousarray(np.asarray(outA, dtype=np.float32))
    outB = np.ascontiguousarray(np.asarray(outB, dtype=np.float32))
    matchA = np.asarray(matchA).astype(np.int64)
    matchB = np.asarray(matchB).astype(np.int64)
    nonMatchA = np.asarray(nonMatchA).astype(np.int64)
    nonMatchB = np.asarray(nonMatchB).astype(np.int64)

    cores = []
    for c in range(NCORES):
        b, h = c // 2, c % 2
        a = np.concatenate(
            [matchA[b, h * MH : (h + 1) * MH], nonMatchA[b, h * NH : (h + 1) * NH]]
        )
        bb = np.concatenate(
            [matchB[b, h * MH : (h + 1) * MH], nonMatchB[b, h * NH : (h + 1) * NH]]
        )
        ismatch = np.zeros(NS, np.bool_)
        ismatch[:MH] = True
        pairsA, pairsB = _plan(a, bb)
        cores.append((a, bb, ismatch, pairsA, pairsB))

    # shared kernel shape: full pair instructions only, min across cores
    nPA = min(len(pa) for _, _, _, pa, _ in cores) // P
    nPB = min(len(pb) for _, _, _, _, pb in cores) // P
    n_in_pairs = 2 * P * (nPA + nPB)
    nScols = -(-(NS - n_in_pairs) // P)  # leftover pairs spill into singles
    COLS = 2 * nPA + 2 * nPB + nScols

    maps = []
    for ci, (a, bb, ismatch, pairsA, pairsB) in enumerate(cores):
        b = ci // 2
        ia = np.zeros((P, COLS), np.int32)
        ib = np.zeros((P, COLS), np.int32)
        wm = np.zeros((P, COLS), np.float32)
        wn = np.zeros((P, COLS), np.float32)
        used = np.zeros(NS, np.bool_)

        def place(s, p, col):
            ia[p, col] = a[s]
            ib[p, col] = bb[s]
            wm[p, col] = 1.0 if ismatch[s] else 0.0
            wn[p, col] = 0.0 if ismatch[s] else 1.0
            used[s] = True

        for t in range(nPA * P):
            s0, s1 = pairsA[t]
            k, p = divmod(t, P)
            place(s0, p, 2 * k)
            place(s1, p, 2 * k + 1)
        for t in range(nPB * P):
            s0, s1 = pairsB[t]
            k, p = divmod(t, P)
            place(s0, p, 2 * nPA + 2 * k)
            place(s1, p, 2 * nPA + 2 * k + 1)
        singles = np.where(~used)[0]
        base = 2 * nPA + 2 * nPB
        for i, s in enumerate(singles):
            place(s, i % P, base + i // P)

        # pair-region invariants (upper row = lower row + 1)
        for k in range(nPA):
            assert np.all(ia[:, 2 * k + 1] == ia[:, 2 * k] + 1)
        for k in range(nPB):
            cc = 2 * nPA + 2 * k
            assert np.all(ib[:, cc + 1] == ib[:, cc] + 1)

        maps.append(
            {
                "eA": outA[b],
                "eB": outB[b],
                "ia": ia,
                "ib": ib,
                "wm": wm,
                "wn": wn,
            }
        )
    return maps, nPA, nPB, COLS


def kernel(outA, outB, matchA, matchB, nonMatchA, nonMatchB):
    global LAST_EXEC_NS
    from concourse import bass_utils

    maps, nPA, nPB, COLS = _in_maps(
        outA, outB, matchA, matchB, nonMatchA, nonMatchB
    )
    ck = (nPA, nPB, COLS)
    if _CACHE.get("key") != ck:
        _CACHE["nc"] = _build_nc(nPA, nPB, COLS)
        _CACHE["key"] = ck
    nc = _CACHE["nc"]

    kwargs = {}
    if os.environ.get("KERNEL_TRACE", "0") == "1":
        kwargs["trace"] = True
    r = bass_utils.run_bass_kernel_spmd(
        nc, maps, core_ids=list(range(NCORES)), **kwargs
    )
    LAST_EXEC_NS = r.exec_time_ns

    partial = np.stack(
        [np.asarray(r.results[c]["out"]).ravel() for c in range(NCORES)]
    )
    match_loss = partial[:, 0].sum(dtype=np.float64) / M_MATCH
    nonmatch_loss = (
        NON_MATCH_WEIGHT * partial[:, 1].sum(dtype=np.float64) / M_NONMATCH
    )
    contrastive = match_loss + nonmatch_loss
    return (
        np.float32(contrastive),
        np.float32(match_loss),
        np.float32(nonmatch_loss),
    )


# revision 17
# speedup vs baseline: 1.2992x; 1.0323x over previous
"""ContrastiveLoss Trainium2 kernel — adjacency-paired gathers.

Same data-parallel layout as kernel.py (8 cores = 4 batches x 2 halves,
27500 sample pairs per core), same indirect1d gather primitive (one index
per SBUF partition per instruction, streaming the out free dim contiguously
from that row — hardware-verified semantics).

New: the per-instruction cost (~1.4 us) is independent of the streamed
length, so samples whose A-rows (or B-rows) are CONSECUTIVE in DRAM are
host-paired and fetched by one C=2 descriptor streaming 2 rows (128 B) into
two adjacent slot columns. Greedy pairing on the A side, then on the B side
among the rest, removes (pA+pB)/128 ~ 25 of the 432 gather instructions.

Slot layout per core (COLS columns x 128 partitions):
  [A-pair region: 2*nPA cols][B-pair region: 2*nPB cols][singles + pads]
A-pair instruction k reads ia[:, 2k] and writes columns 2k, 2k+1 (the host
guarantees ia[p, 2k+1] == ia[p, 2k]+1); its B side uses normal C=1 columns.
Per-slot f32 weights wm/wn (1.0 for match/nonmatch, 0 for pads) replace the
old block masks: dist -> match partial = sum dist*wm, nonmatch partial =
sum relu(0.5-dist)*wn, partition-reduced by a ones-vector matmul.
"""

import os

import numpy as np

B, N, D = 4, 307200, 16
M_MATCH, M_NONMATCH = 5000, 50000
MARGIN = 0.5
NON_MATCH_WEIGHT = 1.0
NCORES = 8

P = 128
MH = M_MATCH // 2
NH = M_NONMATCH // 2
NS = MH + NH  # 27500 samples per core
CHUNK = 32  # compute-chunk width in columns (even, so pairs never straddle)

LAST_EXEC_NS = None
_CACHE = {}


def _pair_scan(vals, order, gap):
    """Greedy pairing over `order` (sorted by vals): pair consecutive sorted
    entries whose values differ by exactly `gap`."""
    pairs = []
    i = 0
    while i < len(order) - 1:
        s0, s1 = order[i], order[i + 1]
        if vals[s1] == vals[s0] + gap:
            pairs.append((s0, s1))
            i += 2
        else:
            i += 1
    return pairs


def _plan(a, b):
    """Pair samples greedily: gap-1 on A, gap-1 on B, gap-2 on A, gap-2 on B
    (each round runs on the samples the earlier rounds left unused)."""
    n = len(a)
    used = np.zeros(n, np.bool_)

    def round_(vals, gap):
        rem = np.where(~used)[0]
        order = rem[np.argsort(vals[rem], kind="stable")]
        pairs = _pair_scan(vals, order, gap)
        for s0, s1 in pairs:
            used[s0] = used[s1] = True
        return pairs

    pA1 = round_(a, 1)
    pB1 = round_(b, 1)
    pA2 = round_(a, 2)
    pB2 = round_(b, 2)
    return pA1, pB1, pA2, pB2


def _build_nc(n1, n2, n3, n4, COLS):
    import concourse.bacc as bacc
    import concourse.mybir as mybir
    import concourse.tile as tile
    from concourse import bass

    f32 = mybir.dt.float32
    i32 = mybir.dt.int32
    X = mybir.AxisListType.X
    ADD = mybir.AluOpType.add
    MULT = mybir.AluOpType.mult
    Relu = mybir.ActivationFunctionType.Relu

    nc = bacc.Bacc(
        "TRN2", target_bir_lowering=False, debug=False, num_swdge_queues=2
    )
    eA = nc.dram_tensor("eA", (N, D), f32, kind="ExternalInput")
    eB = nc.dram_tensor("eB", (N, D), f32, kind="ExternalInput")
    ia = nc.dram_tensor("ia", (P, COLS), i32, kind="ExternalInput")
    ib = nc.dram_tensor("ib", (P, COLS), i32, kind="ExternalInput")
    wm = nc.dram_tensor("wm", (P, COLS), f32, kind="ExternalInput")
    wn = nc.dram_tensor("wn", (P, COLS), f32, kind="ExternalInput")
    out = nc.dram_tensor("out", (1, 2), f32, kind="ExternalOutput")

    qctr = [0]

    def gather(dst_ap, src, idx_ap):
        inst = nc.gpsimd.indirect_dma_start(
            out=dst_ap,
            out_offset=None,
            in_=src.ap(),
            in_offset=bass.IndirectOffsetOnAxis(ap=idx_ap, axis=0),
        )
        if qctr[0] % 2:
            inst.ins.queue = "qPoolDynamic1"
        qctr[0] += 1

    with tile.TileContext(nc) as tc:
        with (
            tc.tile_pool(name="io", bufs=1) as iop,
            tc.tile_pool(name="gath", bufs=1) as gp,
            tc.tile_pool(name="cmp", bufs=4) as cp,
            tc.tile_pool(name="psum", bufs=1, space="PSUM") as pp,
        ):
            # first compute chunk's indices load first
            c0 = min(CHUNK, COLS)
            ia_t = iop.tile([P, COLS], i32)
            nc.sync.dma_start(ia_t[:, :c0], ia.ap()[:, :c0])
            ib_t = iop.tile([P, COLS], i32)
            nc.sync.dma_start(ib_t[:, :c0], ib.ap()[:, :c0])
            if COLS > c0:
                nc.sync.dma_start(ia_t[:, c0:], ia.ap()[:, c0:])
                nc.sync.dma_start(ib_t[:, c0:], ib.ap()[:, c0:])
            wm_t = iop.tile([P, COLS], f32)
            nc.sync.dma_start(wm_t[:], wm.ap())
            wn_t = iop.tile([P, COLS], f32)
            nc.sync.dma_start(wn_t[:], wn.ap())
            margin_t = iop.tile([P, 1], f32)
            nc.vector.memset(margin_t[:], MARGIN)

            gA = gp.tile([P, COLS * D], f32)
            gB = gp.tile([P, COLS * D], f32)
            dist = gp.tile([P, COLS], f32)
            hng = gp.tile([P, COLS], f32)

            b1b = 2 * n1
            a2b = b1b + 2 * n2
            b2b = a2b + 3 * n3
            sb = b2b + 3 * n4

            # gap-2 pair blocks leave their middle cell unwritten on the
            # single-descriptor side; zero those cells so stale SBUF can't
            # poison the (weight-0) distance with NaN/Inf.
            if n3:
                nc.vector.memset(
                    gB[:, a2b * D : (a2b + 3 * n3) * D].rearrange(
                        "p (m c) -> p m c", c=3 * D
                    )[:, :, D : 2 * D],
                    0.0,
                )
            if n4:
                nc.vector.memset(
                    gA[:, b2b * D : (b2b + 3 * n4) * D].rearrange(
                        "p (m c) -> p m c", c=3 * D
                    )[:, :, D : 2 * D],
                    0.0,
                )

            def emit_col(c):
                if c < b1b:  # A gap-1 pairs: C=2 descriptor covers 2 cols
                    if c % 2 == 0:
                        gather(gA[:, c * D : (c + 2) * D], eA, ia_t[:, c : c + 1])
                    gather(gB[:, c * D : (c + 1) * D], eB, ib_t[:, c : c + 1])
                elif c < a2b:  # B gap-1 pairs
                    if (c - b1b) % 2 == 0:
                        gather(gB[:, c * D : (c + 2) * D], eB, ib_t[:, c : c + 1])
                    gather(gA[:, c * D : (c + 1) * D], eA, ia_t[:, c : c + 1])
                elif c < b2b:  # A gap-2 pairs: C=3, middle col wasted
                    loc = (c - a2b) % 3
                    if loc == 0:
                        gather(gA[:, c * D : (c + 3) * D], eA, ia_t[:, c : c + 1])
                    if loc != 1:
                        gather(gB[:, c * D : (c + 1) * D], eB, ib_t[:, c : c + 1])
                elif c < sb:  # B gap-2 pairs
                    loc = (c - b2b) % 3
                    if loc == 0:
                        gather(gB[:, c * D : (c + 3) * D], eB, ib_t[:, c : c + 1])
                    if loc != 1:
                        gather(gA[:, c * D : (c + 1) * D], eA, ia_t[:, c : c + 1])
                else:  # singles
                    gather(gA[:, c * D : (c + 1) * D], eA, ia_t[:, c : c + 1])
                    gather(gB[:, c * D : (c + 1) * D], eB, ib_t[:, c : c + 1])

            for cs in range(0, COLS, CHUNK):
                ce = min(cs + CHUNK, COLS)
                for c in range(cs, ce):
                    emit_col(c)
                w = ce - cs
                nd = cp.tile([P, CHUNK * D], f32, tag="nd")
                nc.vector.tensor_sub(
                    nd[:, : w * D], gA[:, cs * D : ce * D], gB[:, cs * D : ce * D]
                )
                nsq = cp.tile([P, CHUNK * D], f32, tag="nsq")
                nc.scalar.square(nsq[:, : w * D], nd[:, : w * D])
                nc.vector.tensor_reduce(
                    dist[:, cs:ce],
                    nsq[:, : w * D].rearrange("p (s d) -> p s d", d=D),
                    axis=X,
                    op=ADD,
                )
                nc.scalar.activation(
                    hng[:, cs:ce],
                    dist[:, cs:ce],
                    Relu,
                    bias=margin_t[:],
                    scale=-1.0,
                )

            # weighted partials + cross-partition reduction
            sums = gp.tile([P, 2], f32)
            md = cp.tile([P, COLS], f32, tag="md")
            nc.vector.tensor_tensor(out=md[:], in0=dist[:], in1=wm_t[:], op=MULT)
            nc.vector.tensor_reduce(sums[:, 0:1], md[:], axis=X, op=ADD)
            nh = cp.tile([P, COLS], f32, tag="nh")
            nc.vector.tensor_tensor(out=nh[:], in0=hng[:], in1=wn_t[:], op=MULT)
            nc.vector.tensor_reduce(sums[:, 1:2], nh[:], axis=X, op=ADD)

            ones = gp.tile([P, 1], f32)
            nc.vector.memset(ones[:], 1.0)
            acc = pp.tile([1, 2], f32, space="PSUM")
            nc.tensor.matmul(acc[:], lhsT=ones[:], rhs=sums[:], start=True, stop=True)
            res = gp.tile([1, 2], f32)
            nc.vector.tensor_copy(res[:], acc[:])
            nc.sync.dma_start(out.ap(), res[:])

    nc.compile()
    return nc


def _in_maps(outA, outB, matchA, matchB, nonMatchA, nonMatchB):
    outA = np.ascontiguousarray(np.asarray(outA, dtype=np.float32))
    outB = np.ascontiguousarray(np.asarray(outB, dtype=np.float32))
    matchA = np.asarray(matchA).astype(np.int64)
    matchB = np.asarray(matchB).astype(np.int64)
    nonMatchA = np.asarray(nonMatchA).astype(np.int64)
    nonMatchB = np.asarray(nonMatchB).astype(np.int64)

    cores = []
    for c in range(NCORES):
        b, h = c // 2, c % 2
        a = np.concatenate(
            [matchA[b, h * MH : (h + 1) * MH], nonMatchA[b, h * NH : (h + 1) * NH]]
        )
        bb = np.concatenate(
            [matchB[b, h * MH : (h + 1) * MH], nonMatchB[b, h * NH : (h + 1) * NH]]
        )
        ismatch = np.zeros(NS, np.bool_)
        ismatch[:MH] = True
        plists = _plan(a, bb)
        cores.append((a, bb, ismatch, plists))

    # shared kernel shape: full pair instructions only, min across cores
    n1, n2, n3, n4 = (
        min(len(core[3][r]) for core in cores) // P for r in range(4)
    )
    n_in_pairs = 2 * P * (n1 + n2 + n3 + n4)
    nScols = -(-(NS - n_in_pairs) // P)  # leftover pairs spill into singles
    COLS = 2 * n1 + 2 * n2 + 3 * n3 + 3 * n4 + nScols

    maps = []
    for ci, (a, bb, ismatch, plists) in enumerate(cores):
        b = ci // 2
        ia = np.zeros((P, COLS), np.int32)
        ib = np.zeros((P, COLS), np.int32)
        wm = np.zeros((P, COLS), np.float32)
        wn = np.zeros((P, COLS), np.float32)
        used = np.zeros(NS, np.bool_)

        def place(s, p, col):
            ia[p, col] = a[s]
            ib[p, col] = bb[s]
            wm[p, col] = 1.0 if ismatch[s] else 0.0
            wn[p, col] = 0.0 if ismatch[s] else 1.0
            used[s] = True

        pA1, pB1, pA2, pB2 = plists
        b1b = 2 * n1
        a2b = b1b + 2 * n2
        b2b = a2b + 3 * n3
        sbase = b2b + 3 * n4
        for t in range(n1 * P):
            s0, s1 = pA1[t]
            k, p = divmod(t, P)
            place(s0, p, 2 * k)
            place(s1, p, 2 * k + 1)
        for t in range(n2 * P):
            s0, s1 = pB1[t]
            k, p = divmod(t, P)
            place(s0, p, b1b + 2 * k)
            place(s1, p, b1b + 2 * k + 1)
        for t in range(n3 * P):
            s0, s1 = pA2[t]
            k, p = divmod(t, P)
            place(s0, p, a2b + 3 * k)
            place(s1, p, a2b + 3 * k + 2)
        for t in range(n4 * P):
            s0, s1 = pB2[t]
            k, p = divmod(t, P)
            place(s0, p, b2b + 3 * k)
            place(s1, p, b2b + 3 * k + 2)
        singles = np.where(~used)[0]
        for i, s in enumerate(singles):
            place(s, i % P, sbase + i // P)

        # pair-region invariants (upper row = lower row + gap)
        for k in range(n1):
            assert np.all(ia[:, 2 * k + 1] == ia[:, 2 * k] + 1)
        for k in range(n2):
            cc = b1b + 2 * k
            assert np.all(ib[:, cc + 1] == ib[:, cc] + 1)
        for k in range(n3):
            cc = a2b + 3 * k
            assert np.all(ia[:, cc + 2] == ia[:, cc] + 2)
        for k in range(n4):
            cc = b2b + 3 * k
            assert np.all(ib[:, cc + 2] == ib[:, cc] + 2)

        maps.append(
            {
                "eA": outA[b],
                "eB": outB[b],
                "ia": ia,
                "ib": ib,
                "wm": wm,
                "wn": wn,
            }
        )
    return maps, n1, n2, n3, n4, COLS


def kernel(outA, outB, matchA, matchB, nonMatchA, nonMatchB):
    global LAST_EXEC_NS
    from concourse import bass_utils

    maps, n1, n2, n3, n4, COLS = _in_maps(
        outA, outB, matchA, matchB, nonMatchA, nonMatchB
    )
    ck = (n1, n2, n3, n4, COLS)
    if _CACHE.get("key") != ck:
        _CACHE["nc"] = _build_nc(n1, n2, n3, n4, COLS)
        _CACHE["key"] = ck
    nc = _CACHE["nc"]

    kwargs = {}
    if os.environ.get("KERNEL_TRACE", "0") == "1":
        kwargs["trace"] = True
    r = bass_utils.run_bass_kernel_spmd(
        nc, maps, core_ids=list(range(NCORES)), **kwargs
    )
    LAST_EXEC_NS = r.exec_time_ns

    partial = np.stack(
        [np.asarray(r.results[c]["out"]).ravel() for c in range(NCORES)]
    )
    match_loss = partial[:, 0].sum(dtype=np.float64) / M_MATCH
    nonmatch_loss = (
        NON_MATCH_WEIGHT * partial[:, 1].sum(dtype=np.float64) / M_NONMATCH
    )
    contrastive = match_loss + nonmatch_loss
    return (
        np.float32(contrastive),
        np.float32(match_loss),
        np.float32(nonmatch_loss),
    )


# revision 20
# speedup vs baseline: 1.3649x; 1.0506x over previous
"""ContrastiveLoss Trainium2 kernel — adjacency-paired gathers.

Same data-parallel layout as kernel.py (8 cores = 4 batches x 2 halves,
27500 sample pairs per core), same indirect1d gather primitive (one index
per SBUF partition per instruction, streaming the out free dim contiguously
from that row — hardware-verified semantics).

New: the per-instruction cost (~1.4 us) is independent of the streamed
length, so samples whose A-rows (or B-rows) lie at DRAM distance 1 or 2 are
host-paired and fetched by one descriptor streaming 2 rows (C=2, gap 1) or
3 rows (C=3, gap 2, middle cell wasted). Greedy rounds — gap-1 on A, gap-1
on B, gap-2 on A, gap-2 on B, each on the samples earlier rounds left —
remove ~45 of the 432 naive gather instructions (387 remain).

Slot layout per core (COLS columns x 128 partitions):
  [A1: 2*n1][B1: 2*n2][A2: 3*n3][B2: 3*n4][singles + pads]
A gap-g pair instruction k reads ia[:, c0] and writes columns c0..c0+g (the
host guarantees ia[p, c0+g] == ia[p, c0]+g); the partner side uses normal
C=1 columns at the real cells, and the wasted middle cells of gap-2 blocks
are zeroed by two strided memsets (weights there are 0; the memset keeps
stale SBUF NaN/Inf out of the weighted sum).
Per-slot f32 weights wm/wn (1.0 for match/nonmatch, 0 for pads) replace the
old block masks: dist -> match partial = sum dist*wm, nonmatch partial =
sum relu(0.5-dist)*wn, partition-reduced by a ones-vector matmul.
"""

import os

import numpy as np

B, N, D = 4, 307200, 16
M_MATCH, M_NONMATCH = 5000, 50000
MARGIN = 0.5
NON_MATCH_WEIGHT = 1.0
NCORES = 8

P = 128
MH = M_MATCH // 2
NH = M_NONMATCH // 2
NS = MH + NH  # 27500 samples per core
CHUNK = 32  # compute-chunk width in columns (even, so pairs never straddle)

LAST_EXEC_NS = None
_CACHE = {}


def _pair_scan(vals, order, gap):
    """Greedy pairing over `order` (sorted by vals): pair consecutive sorted
    entries whose values differ by exactly `gap`."""
    pairs = []
    i = 0
    while i < len(order) - 1:
        s0, s1 = order[i], order[i + 1]
        if vals[s1] == vals[s0] + gap:
            pairs.append((s0, s1))
            i += 2
        else:
            i += 1
    return pairs


# pairing rounds: (side, gap); side 0 pairs on A-rows, side 1 on B-rows.
# Each round runs greedily on the samples earlier rounds left unused.
ROUNDS = [(0, 1), (1, 1), (0, 2), (1, 2), (0, 3), (1, 3), (0, 4), (1, 4)]


def _plan(a, b):
    used = np.zeros(len(a), np.bool_)
    out = []
    for side, gap in ROUNDS:
        vals = a if side == 0 else b
        rem = np.where(~used)[0]
        order = rem[np.argsort(vals[rem], kind="stable")]
        pairs = _pair_scan(vals, order, gap)
        for s0, s1 in pairs:
            used[s0] = used[s1] = True
        out.append(pairs)
    return out


def _build_nc(ns, COLS):
    import concourse.bacc as bacc
    import concourse.mybir as mybir
    import concourse.tile as tile
    from concourse import bass

    f32 = mybir.dt.float32
    i32 = mybir.dt.int32
    X = mybir.AxisListType.X
    ADD = mybir.AluOpType.add
    MULT = mybir.AluOpType.mult
    Relu = mybir.ActivationFunctionType.Relu

    nc = bacc.Bacc(
        "TRN2", target_bir_lowering=False, debug=False, num_swdge_queues=2
    )
    eA = nc.dram_tensor("eA", (N, D), f32, kind="ExternalInput")
    eB = nc.dram_tensor("eB", (N, D), f32, kind="ExternalInput")
    ia = nc.dram_tensor("ia", (P, COLS), i32, kind="ExternalInput")
    ib = nc.dram_tensor("ib", (P, COLS), i32, kind="ExternalInput")
    wm = nc.dram_tensor("wm", (P, COLS), f32, kind="ExternalInput")
    wn = nc.dram_tensor("wn", (P, COLS), f32, kind="ExternalInput")
    out = nc.dram_tensor("out", (1, 2), f32, kind="ExternalOutput")

    qctr = [0]

    def gather(dst_ap, src, idx_ap):
        inst = nc.gpsimd.indirect_dma_start(
            out=dst_ap,
            out_offset=None,
            in_=src.ap(),
            in_offset=bass.IndirectOffsetOnAxis(ap=idx_ap, axis=0),
        )
        if qctr[0] % 2:
            inst.ins.queue = "qPoolDynamic1"
        qctr[0] += 1

    with tile.TileContext(nc) as tc:
        with (
            tc.tile_pool(name="io", bufs=1) as iop,
            tc.tile_pool(name="gath", bufs=1) as gp,
            tc.tile_pool(name="cmp", bufs=4) as cp,
            tc.tile_pool(name="psum", bufs=1, space="PSUM") as pp,
        ):
            # first compute chunk's indices load first
            c0 = min(CHUNK, COLS)
            ia_t = iop.tile([P, COLS], i32)
            nc.sync.dma_start(ia_t[:, :c0], ia.ap()[:, :c0])
            ib_t = iop.tile([P, COLS], i32)
            nc.sync.dma_start(ib_t[:, :c0], ib.ap()[:, :c0])
            if COLS > c0:
                nc.sync.dma_start(ia_t[:, c0:], ia.ap()[:, c0:])
                nc.sync.dma_start(ib_t[:, c0:], ib.ap()[:, c0:])
            wm_t = iop.tile([P, COLS], f32)
            nc.sync.dma_start(wm_t[:], wm.ap())
            wn_t = iop.tile([P, COLS], f32)
            nc.sync.dma_start(wn_t[:], wn.ap())
            margin_t = iop.tile([P, 1], f32)
            nc.vector.memset(margin_t[:], MARGIN)

            gA = gp.tile([P, COLS * D], f32)
            gB = gp.tile([P, COLS * D], f32)
            dist = gp.tile([P, COLS], f32)
            hng = gp.tile([P, COLS], f32)

            regions = []  # (start, end, side, gap) in ROUNDS order
            base = 0
            for (side, gap), n in zip(ROUNDS, ns):
                w = gap + 1
                regions.append((base, base + w * n, side, gap))
                base += w * n

            # gap>=2 pair blocks leave interior cells unwritten on the
            # single-descriptor side; zero them so stale SBUF can't poison
            # the (weight-0) distance with NaN/Inf.
            for start, end, side, gap in regions:
                if gap >= 2 and end > start:
                    buf = gB if side == 0 else gA
                    nc.vector.memset(
                        buf[:, start * D : end * D].rearrange(
                            "p (m c) -> p m c", c=(gap + 1) * D
                        )[:, :, D : gap * D],
                        0.0,
                    )

            def emit_col(c):
                for start, end, side, gap in regions:
                    if c < end:
                        loc = (c - start) % (gap + 1)
                        pair_src = (gA, eA, ia_t) if side == 0 else (gB, eB, ib_t)
                        sgl_src = (gB, eB, ib_t) if side == 0 else (gA, eA, ia_t)
                        if loc == 0:
                            g_t, e_t, i_t = pair_src
                            gather(
                                g_t[:, c * D : (c + gap + 1) * D],
                                e_t,
                                i_t[:, c : c + 1],
                            )
                        if loc == 0 or loc == gap:
                            g_t, e_t, i_t = sgl_src
                            gather(
                                g_t[:, c * D : (c + 1) * D], e_t, i_t[:, c : c + 1]
                            )
                        return
                gather(gA[:, c * D : (c + 1) * D], eA, ia_t[:, c : c + 1])
                gather(gB[:, c * D : (c + 1) * D], eB, ib_t[:, c : c + 1])

            for cs in range(0, COLS, CHUNK):
                ce = min(cs + CHUNK, COLS)
                for c in range(cs, ce):
                    emit_col(c)
                w = ce - cs
                nd = cp.tile([P, CHUNK * D], f32, tag="nd")
                nc.vector.tensor_sub(
                    nd[:, : w * D], gA[:, cs * D : ce * D], gB[:, cs * D : ce * D]
                )
                nsq = cp.tile([P, CHUNK * D], f32, tag="nsq")
                nc.scalar.square(nsq[:, : w * D], nd[:, : w * D])
                nc.vector.tensor_reduce(
                    dist[:, cs:ce],
                    nsq[:, : w * D].rearrange("p (s d) -> p s d", d=D),
                    axis=X,
                    op=ADD,
                )
                nc.scalar.activation(
                    hng[:, cs:ce],
                    dist[:, cs:ce],
                    Relu,
                    bias=margin_t[:],
                    scale=-1.0,
                )

            # weighted partials + cross-partition reduction
            sums = gp.tile([P, 2], f32)
            md = cp.tile([P, COLS], f32, tag="md")
            nc.vector.tensor_tensor(out=md[:], in0=dist[:], in1=wm_t[:], op=MULT)
            nc.vector.tensor_reduce(sums[:, 0:1], md[:], axis=X, op=ADD)
            nh = cp.tile([P, COLS], f32, tag="nh")
            nc.vector.tensor_tensor(out=nh[:], in0=hng[:], in1=wn_t[:], op=MULT)
            nc.vector.tensor_reduce(sums[:, 1:2], nh[:], axis=X, op=ADD)

            ones = gp.tile([P, 1], f32)
            nc.vector.memset(ones[:], 1.0)
            acc = pp.tile([1, 2], f32, space="PSUM")
            nc.tensor.matmul(acc[:], lhsT=ones[:], rhs=sums[:], start=True, stop=True)
            res = gp.tile([1, 2], f32)
            nc.vector.tensor_copy(res[:], acc[:])
            nc.sync.dma_start(out.ap(), res[:])

    nc.compile()
    return nc


def _in_maps(outA, outB, matchA, matchB, nonMatchA, nonMatchB):
    outA = np.ascontiguousarray(np.asarray(outA, dtype=np.float32))
    outB = np.ascontiguousarray(np.asarray(outB, dtype=np.float32))
    matchA = np.asarray(matchA).astype(np.int64)
    matchB = np.asarray(matchB).astype(np.int64)
    nonMatchA = np.asarray(nonMatchA).astype(np.int64)
    nonMatchB = np.asarray(nonMatchB).astype(np.int64)

    cores = []
    for c in range(NCORES):
        b, h = c // 2, c % 2
        a = np.concatenate(
            [matchA[b, h * MH : (h + 1) * MH], nonMatchA[b, h * NH : (h + 1) * NH]]
        )
        bb = np.concatenate(
            [matchB[b, h * MH : (h + 1) * MH], nonMatchB[b, h * NH : (h + 1) * NH]]
        )
        ismatch = np.zeros(NS, np.bool_)
        ismatch[:MH] = True
        plists = _plan(a, bb)
        cores.append((a, bb, ismatch, plists))

    # shared kernel shape: full pair instructions only, min across cores
    ns = [
        min(len(core[3][r]) for core in cores) // P for r in range(len(ROUNDS))
    ]
    n_in_pairs = 2 * P * sum(ns)
    nScols = -(-(NS - n_in_pairs) // P)  # leftover pairs spill into singles
    COLS = sum(n * (gap + 1) for n, (_, gap) in zip(ns, ROUNDS)) + nScols

    maps = []
    for ci, (a, bb, ismatch, plists) in enumerate(cores):
        b = ci // 2
        ia = np.zeros((P, COLS), np.int32)
        ib = np.zeros((P, COLS), np.int32)
        wm = np.zeros((P, COLS), np.float32)
        wn = np.zeros((P, COLS), np.float32)
        used = np.zeros(NS, np.bool_)

        def place(s, p, col):
            ia[p, col] = a[s]
            ib[p, col] = bb[s]
            wm[p, col] = 1.0 if ismatch[s] else 0.0
            wn[p, col] = 0.0 if ismatch[s] else 1.0
            used[s] = True

        base = 0
        for r, ((side, gap), n) in enumerate(zip(ROUNDS, ns)):
            w = gap + 1
            pl = plists[r]
            for t in range(n * P):
                s0, s1 = pl[t]
                k, p = divmod(t, P)
                place(s0, p, base + w * k)
                place(s1, p, base + w * k + gap)
            iv = ia if side == 0 else ib
            for k in range(n):
                cc = base + w * k
                assert np.all(iv[:, cc + gap] == iv[:, cc] + gap)
            base += w * n
        singles = np.where(~used)[0]
        for i, s in enumerate(singles):
            place(s, i % P, base + i // P)

        maps.append(
            {
                "eA": outA[b],
                "eB": outB[b],
                "ia": ia,
                "ib": ib,
                "wm": wm,
                "wn": wn,
            }
        )
    return maps, ns, COLS


def kernel(outA, outB, matchA, matchB, nonMatchA, nonMatchB):
    global LAST_EXEC_NS
    from concourse import bass_utils

    maps, ns, COLS = _in_maps(
        outA, outB, matchA, matchB, nonMatchA, nonMatchB
    )
    ck = (tuple(ns), COLS)
    if _CACHE.get("key") != ck:
        _CACHE["nc"] = _build_nc(ns, COLS)
        _CACHE["key"] = ck
    nc = _CACHE["nc"]

    kwargs = {}
    if os.environ.get("KERNEL_TRACE", "0") == "1":
        kwargs["trace"] = True
    r = bass_utils.run_bass_kernel_spmd(
        nc, maps, core_ids=list(range(NCORES)), **kwargs
    )
    LAST_EXEC_NS = r.exec_time_ns

    partial = np.stack(
        [np.asarray(r.results[c]["out"]).ravel() for c in range(NCORES)]
    )
    match_loss = partial[:, 0].sum(dtype=np.float64) / M_MATCH
    nonmatch_loss = (
        NON_MATCH_WEIGHT * partial[:, 1].sum(dtype=np.float64) / M_NONMATCH
    )
    contrastive = match_loss + nonmatch_loss
    return (
        np.float32(contrastive),
        np.float32(match_loss),
        np.float32(nonmatch_loss),
    )


# revision 22
# speedup vs baseline: 1.4112x; 1.0339x over previous
"""ContrastiveLoss Trainium2 kernel — adjacency-paired gathers.

Same data-parallel layout as kernel.py (8 cores = 4 batches x 2 halves,
27500 sample pairs per core), same indirect1d gather primitive (one index
per SBUF partition per instruction, streaming the out free dim contiguously
from that row — hardware-verified semantics).

New: the per-instruction cost (~1.4 us) is independent of the streamed
length, so samples whose A-rows (or B-rows) lie at DRAM distance 1 or 2 are
host-paired and fetched by one descriptor streaming gap+1 rows (interior
cells wasted). Greedy ROUNDS — gaps 1..4, alternating A then B, each on the
samples earlier rounds left unused — remove ~62 of the 432 naive gather
instructions (370 remain).

Slot layout per core (COLS columns x 128 partitions): one region per
(side, gap) round, block width gap+1, then [singles + pads].
A gap-g pair instruction k reads ia[:, c0] and writes columns c0..c0+g (the
host guarantees ia[p, c0+g] == ia[p, c0]+g); the partner side uses normal
C=1 columns at the real cells, and the wasted interior cells of gap>=2 blocks
are zeroed by strided memsets (weights there are 0; the memset keeps
stale SBUF NaN/Inf out of the weighted sum).
Per-slot f32 weights wm/wn (1.0 for match/nonmatch, 0 for pads) replace the
old block masks: dist -> match partial = sum dist*wm, nonmatch partial =
sum relu(0.5-dist)*wn, partition-reduced by a ones-vector matmul.
"""

import os

import numpy as np

B, N, D = 4, 307200, 16
M_MATCH, M_NONMATCH = 5000, 50000
MARGIN = 0.5
NON_MATCH_WEIGHT = 1.0
NCORES = 8

P = 128
MH = M_MATCH // 2
NH = M_NONMATCH // 2
NS = MH + NH  # 27500 samples per core
CHUNK = 32  # compute-chunk width in columns (straddling pair blocks are
# safe: the tile framework tracks dependencies per byte range)

LAST_EXEC_NS = None
_CACHE = {}


def _pair_scan(vals, order, gap):
    """Greedy pairing over `order` (sorted by vals): pair consecutive sorted
    entries whose values differ by exactly `gap`."""
    pairs = []
    i = 0
    while i < len(order) - 1:
        s0, s1 = order[i], order[i + 1]
        if vals[s1] == vals[s0] + gap:
            pairs.append((s0, s1))
            i += 2
        else:
            i += 1
    return pairs


# pairing rounds: (side, gap); side 0 pairs on A-rows, side 1 on B-rows.
# Each round runs greedily on the samples earlier rounds left unused.
ROUNDS = [(side, gap) for gap in range(1, 9) for side in (0, 1)]


def _plan(a, b):
    used = np.zeros(len(a), np.bool_)
    out = []
    for side, gap in ROUNDS:
        vals = a if side == 0 else b
        rem = np.where(~used)[0]
        order = rem[np.argsort(vals[rem], kind="stable")]
        pairs = _pair_scan(vals, order, gap)
        for s0, s1 in pairs:
            used[s0] = used[s1] = True
        out.append(pairs)
    return out


def _build_nc(ns, COLS):
    import concourse.bacc as bacc
    import concourse.mybir as mybir
    import concourse.tile as tile
    from concourse import bass

    f32 = mybir.dt.float32
    i32 = mybir.dt.int32
    X = mybir.AxisListType.X
    ADD = mybir.AluOpType.add
    MULT = mybir.AluOpType.mult
    Relu = mybir.ActivationFunctionType.Relu

    nc = bacc.Bacc(
        "TRN2", target_bir_lowering=False, debug=False, num_swdge_queues=2
    )
    eA = nc.dram_tensor("eA", (N, D), f32, kind="ExternalInput")
    eB = nc.dram_tensor("eB", (N, D), f32, kind="ExternalInput")
    ia = nc.dram_tensor("ia", (P, COLS), i32, kind="ExternalInput")
    ib = nc.dram_tensor("ib", (P, COLS), i32, kind="ExternalInput")
    wm = nc.dram_tensor("wm", (P, COLS), f32, kind="ExternalInput")
    wn = nc.dram_tensor("wn", (P, COLS), f32, kind="ExternalInput")
    out = nc.dram_tensor("out", (1, 2), f32, kind="ExternalOutput")

    qctr = [0]

    def gather(dst_ap, src, idx_ap):
        inst = nc.gpsimd.indirect_dma_start(
            out=dst_ap,
            out_offset=None,
            in_=src.ap(),
            in_offset=bass.IndirectOffsetOnAxis(ap=idx_ap, axis=0),
        )
        if qctr[0] % 2:
            inst.ins.queue = "qPoolDynamic1"
        qctr[0] += 1

    with tile.TileContext(nc) as tc:
        with (
            tc.tile_pool(name="io", bufs=1) as iop,
            tc.tile_pool(name="gath", bufs=1) as gp,
            tc.tile_pool(name="cmp", bufs=4) as cp,
            tc.tile_pool(name="psum", bufs=1, space="PSUM") as pp,
        ):
            # first compute chunk's indices load first
            c0 = min(CHUNK, COLS)
            ia_t = iop.tile([P, COLS], i32)
            nc.sync.dma_start(ia_t[:, :c0], ia.ap()[:, :c0])
            ib_t = iop.tile([P, COLS], i32)
            nc.sync.dma_start(ib_t[:, :c0], ib.ap()[:, :c0])
            if COLS > c0:
                nc.sync.dma_start(ia_t[:, c0:], ia.ap()[:, c0:])
                nc.sync.dma_start(ib_t[:, c0:], ib.ap()[:, c0:])
            wm_t = iop.tile([P, COLS], f32)
            nc.sync.dma_start(wm_t[:], wm.ap())
            wn_t = iop.tile([P, COLS], f32)
            nc.sync.dma_start(wn_t[:], wn.ap())
            margin_t = iop.tile([P, 1], f32)
            nc.vector.memset(margin_t[:], MARGIN)

            gA = gp.tile([P, COLS * D], f32)
            gB = gp.tile([P, COLS * D], f32)
            dist = gp.tile([P, COLS], f32)
            hng = gp.tile([P, COLS], f32)

            regions = []  # (start, end, side, gap) in ROUNDS order
            base = 0
            for (side, gap), n in zip(ROUNDS, ns):
                w = gap + 1
                regions.append((base, base + w * n, side, gap))
                base += w * n

            # gap>=2 pair blocks leave interior cells unwritten on the
            # single-descriptor side; zero them so stale SBUF can't poison
            # the (weight-0) distance with NaN/Inf.
            for start, end, side, gap in regions:
                if gap >= 2 and end > start:
                    buf = gB if side == 0 else gA
                    nc.vector.memset(
                        buf[:, start * D : end * D].rearrange(
                            "p (m c) -> p m c", c=(gap + 1) * D
                        )[:, :, D : gap * D],
                        0.0,
                    )

            def emit_col(c):
                for start, end, side, gap in regions:
                    if c < end:
                        loc = (c - start) % (gap + 1)
                        pair_src = (gA, eA, ia_t) if side == 0 else (gB, eB, ib_t)
                        sgl_src = (gB, eB, ib_t) if side == 0 else (gA, eA, ia_t)
                        if loc == 0:
                            g_t, e_t, i_t = pair_src
                            gather(
                                g_t[:, c * D : (c + gap + 1) * D],
                                e_t,
                                i_t[:, c : c + 1],
                            )
                        if loc == 0 or loc == gap:
                            g_t, e_t, i_t = sgl_src
                            gather(
                                g_t[:, c * D : (c + 1) * D], e_t, i_t[:, c : c + 1]
                            )
                        return
                gather(gA[:, c * D : (c + 1) * D], eA, ia_t[:, c : c + 1])
                gather(gB[:, c * D : (c + 1) * D], eB, ib_t[:, c : c + 1])

            for cs in range(0, COLS, CHUNK):
                ce = min(cs + CHUNK, COLS)
                for c in range(cs, ce):
                    emit_col(c)
                w = ce - cs
                nd = cp.tile([P, CHUNK * D], f32, tag="nd")
                nc.vector.tensor_sub(
                    nd[:, : w * D], gA[:, cs * D : ce * D], gB[:, cs * D : ce * D]
                )
                nsq = cp.tile([P, CHUNK * D], f32, tag="nsq")
                nc.scalar.square(nsq[:, : w * D], nd[:, : w * D])
                nc.vector.tensor_reduce(
                    dist[:, cs:ce],
                    nsq[:, : w * D].rearrange("p (s d) -> p s d", d=D),
                    axis=X,
                    op=ADD,
                )
                nc.scalar.activation(
                    hng[:, cs:ce],
                    dist[:, cs:ce],
                    Relu,
                    bias=margin_t[:],
                    scale=-1.0,
                )

            # weighted partials + cross-partition reduction
            sums = gp.tile([P, 2], f32)
            md = cp.tile([P, COLS], f32, tag="md")
            nc.vector.tensor_tensor(out=md[:], in0=dist[:], in1=wm_t[:], op=MULT)
            nc.vector.tensor_reduce(sums[:, 0:1], md[:], axis=X, op=ADD)
            nh = cp.tile([P, COLS], f32, tag="nh")
            nc.vector.tensor_tensor(out=nh[:], in0=hng[:], in1=wn_t[:], op=MULT)
            nc.vector.tensor_reduce(sums[:, 1:2], nh[:], axis=X, op=ADD)

            ones = gp.tile([P, 1], f32)
            nc.vector.memset(ones[:], 1.0)
            acc = pp.tile([1, 2], f32, space="PSUM")
            nc.tensor.matmul(acc[:], lhsT=ones[:], rhs=sums[:], start=True, stop=True)
            res = gp.tile([1, 2], f32)
            nc.vector.tensor_copy(res[:], acc[:])
            nc.sync.dma_start(out.ap(), res[:])

    nc.compile()
    return nc


def _in_maps(outA, outB, matchA, matchB, nonMatchA, nonMatchB):
    outA = np.ascontiguousarray(np.asarray(outA, dtype=np.float32))
    outB = np.ascontiguousarray(np.asarray(outB, dtype=np.float32))
    matchA = np.asarray(matchA).astype(np.int64)
    matchB = np.asarray(matchB).astype(np.int64)
    nonMatchA = np.asarray(nonMatchA).astype(np.int64)
    nonMatchB = np.asarray(nonMatchB).astype(np.int64)

    cores = []
    for c in range(NCORES):
        b, h = c // 2, c % 2
        a = np.concatenate(
            [matchA[b, h * MH : (h + 1) * MH], nonMatchA[b, h * NH : (h + 1) * NH]]
        )
        bb = np.concatenate(
            [matchB[b, h * MH : (h + 1) * MH], nonMatchB[b, h * NH : (h + 1) * NH]]
        )
        ismatch = np.zeros(NS, np.bool_)
        ismatch[:MH] = True
        plists = _plan(a, bb)
        cores.append((a, bb, ismatch, plists))

    # shared kernel shape: full pair instructions only, min across cores
    ns = [
        min(len(core[3][r]) for core in cores) // P for r in range(len(ROUNDS))
    ]
    n_in_pairs = 2 * P * sum(ns)
    nScols = -(-(NS - n_in_pairs) // P)  # leftover pairs spill into singles
    COLS = sum(n * (gap + 1) for n, (_, gap) in zip(ns, ROUNDS)) + nScols

    maps = []
    for ci, (a, bb, ismatch, plists) in enumerate(cores):
        b = ci // 2
        ia = np.zeros((P, COLS), np.int32)
        ib = np.zeros((P, COLS), np.int32)
        wm = np.zeros((P, COLS), np.float32)
        wn = np.zeros((P, COLS), np.float32)
        used = np.zeros(NS, np.bool_)

        def place(s, p, col):
            ia[p, col] = a[s]
            ib[p, col] = bb[s]
            wm[p, col] = 1.0 if ismatch[s] else 0.0
            wn[p, col] = 0.0 if ismatch[s] else 1.0
            used[s] = True

        base = 0
        for r, ((side, gap), n) in enumerate(zip(ROUNDS, ns)):
            w = gap + 1
            pl = plists[r]
            for t in range(n * P):
                s0, s1 = pl[t]
                k, p = divmod(t, P)
                place(s0, p, base + w * k)
                place(s1, p, base + w * k + gap)
            iv = ia if side == 0 else ib
            for k in range(n):
                cc = base + w * k
                assert np.all(iv[:, cc + gap] == iv[:, cc] + gap)
            base += w * n
        singles = np.where(~used)[0]
        for i, s in enumerate(singles):
            place(s, i % P, base + i // P)

        maps.append(
            {
                "eA": outA[b],
                "eB": outB[b],
                "ia": ia,
                "ib": ib,
                "wm": wm,
                "wn": wn,
            }
        )
    return maps, ns, COLS


def kernel(outA, outB, matchA, matchB, nonMatchA, nonMatchB):
    global LAST_EXEC_NS
    from concourse import bass_utils

    maps, ns, COLS = _in_maps(
        outA, outB, matchA, matchB, nonMatchA, nonMatchB
    )
    ck = (tuple(ns), COLS)
    if _CACHE.get("key") != ck:
        _CACHE["nc"] = _build_nc(ns, COLS)
        _CACHE["key"] = ck
    nc = _CACHE["nc"]

    kwargs = {}
    if os.environ.get("KERNEL_TRACE", "0") == "1":
        kwargs["trace"] = True
    r = bass_utils.run_bass_kernel_spmd(
        nc, maps, core_ids=list(range(NCORES)), **kwargs
    )
    LAST_EXEC_NS = r.exec_time_ns

    partial = np.stack(
        [np.asarray(r.results[c]["out"]).ravel() for c in range(NCORES)]
    )
    match_loss = partial[:, 0].sum(dtype=np.float64) / M_MATCH
    nonmatch_loss = (
        NON_MATCH_WEIGHT * partial[:, 1].sum(dtype=np.float64) / M_NONMATCH
    )
    contrastive = match_loss + nonmatch_loss
    return (
        np.float32(contrastive),
        np.float32(match_loss),
        np.float32(nonmatch_loss),
    )
